# revision 5
# baseline (speedup 1.0000x reference)
"""Trainium2 Bass kernel for nn_DenseProduct (num_factors=2).

Computes, for input x of shape (128, 16, 64, 32) f32:
    out[s, d, b, i*32+j] = x[2s, d, b, i] + x[2s+1, d, b, j]
with output shape (64, 16, 64, 1024) f32.

Sharding: scope axis (dim 0) across 8 NeuronCores — core c gets input
scopes [16c, 16c+16) and produces output scopes [8c, 8c+8), a contiguous
33.5 MB slice of the output per core.

Per-core layout: SBUF partition p = d*8 + b_hi (d in [0,16), b_hi in [0,8),
b = 8*b_hi + b_lo). This makes the input DMA read contiguous 1 KB runs and
the output DMA write one contiguous 4 MB region per scope (32 KB per
partition). The whole outer-sum for one scope is a single DVE tensor_tensor
with stride-0 (broadcast) free dims:
    out[p, (bl, i, j)] = A[p, (bl, i)] + B[p, (bl, j)]
"""

import numpy as np

_S_IN = 128        # total input scopes
_NF = 2            # num_factors (hardcoded)
_S_OUT = _S_IN // _NF
_D = 16
_B = 64
_N = 32
_N_CORES = 8
_SIN_LOC = _S_IN // _N_CORES   # 16 input scopes per core
_S_LOC = _S_OUT // _N_CORES    # 8 output scopes per core
_P = 128
_BH = 8
_BL = 8
_FREE_IN = _BL * _N            # 256
_FREE_OUT = _BL * _N * _N      # 8192

_CACHE = {}
LAST_RESULTS = None  # BassKernelResults of the most recent run (for profiling)


def _build_bass():
    import concourse.bacc as bacc
    import concourse.mybir as mybir
    from concourse.tile import TileContext

    nc = bacc.Bacc("TRN2", target_bir_lowering=False, debug=False,
                   num_devices=_N_CORES)
    x = nc.dram_tensor("x", [_SIN_LOC, _D, _B, _N], mybir.dt.float32,
                       kind="ExternalInput").ap()
    out = nc.dram_tensor("out", [_S_LOC, _D, _B, _N * _N], mybir.dt.float32,
                         kind="ExternalOutput").ap()

    with TileContext(nc) as tc:
        with tc.tile_pool(name="inp", bufs=_S_LOC) as in_pool, \
             tc.tile_pool(name="outp", bufs=3) as out_pool:
            # x[s_in, d, 8*bh+bl, n] -> partition (d, bh), free (s_in, bl, n)
            xr = x.rearrange("s d (bh bl) n -> (d bh) s (bl n)", bh=_BH)
            in_tiles = []
            for s in range(_S_LOC):
                # both factors (s_in = 2s, 2s+1) in one DMA -> one wait sem
                t = in_pool.tile([_P, 2 * _FREE_IN], mybir.dt.float32)
                src = xr[:, 2 * s:2 * s + 2]  # (128, 2, 256), s-stride 32768
                dst = t[:, :].rearrange("p (s f) -> p s f", s=2)
                nc.sync.dma_start(out=dst, in_=src)
                in_tiles.append(t)

            for s in range(_S_LOC):
                ot = out_pool.tile([_P, _FREE_OUT], mybir.dt.float32)
                a = in_tiles[s][:, :_FREE_IN].rearrange("p (bl i) -> p bl i", bl=_BL)
                b = in_tiles[s][:, _FREE_IN:].rearrange("p (bl j) -> p bl j", bl=_BL)
                a4 = a.unsqueeze(3).broadcast_to([_P, _BL, _N, _N])
                b4 = b.unsqueeze(2).broadcast_to([_P, _BL, _N, _N])
                o4 = ot[:, :].rearrange("p (bl i j) -> p bl i j", bl=_BL, i=_N)
                nc.vector.tensor_add(o4, a4, b4)
                dst = out[s].rearrange("d (bh bl) f -> (d bh) (bl f)", bh=_BH)
                nc.sync.dma_start(out=dst, in_=ot[:, :])
    nc.compile()
    return nc


def kernel(x, num_factors):
    global LAST_RESULTS
    from concourse.bass_utils import run_bass_kernel_spmd

    x = np.asarray(x)
    assert x.shape == (_S_IN, _D, _B, _N), x.shape
    assert int(num_factors) == _NF, num_factors
    x = x.astype(np.float32, copy=False)

    if "nc" not in _CACHE:
        _CACHE["nc"] = _build_bass()
    nc = _CACHE["nc"]

    in_maps = [
        {"x": np.ascontiguousarray(x[c * _SIN_LOC:(c + 1) * _SIN_LOC])}
        for c in range(_N_CORES)
    ]
    res = run_bass_kernel_spmd(nc, in_maps, core_ids=list(range(_N_CORES)))
    LAST_RESULTS = res
    out = np.concatenate([res.results[c]["out"] for c in range(_N_CORES)], axis=0)
    return out.reshape(_S_OUT, _D, _B, _N ** _NF)
